# revision 13
# baseline (speedup 1.0000x reference)
"""AdaFace loss on 8 TRN2 NeuronCores, class-parallel.

Strategy: shard the 100k weight rows (classes) across 8 cores. Host
pre-normalizes rows, transposes to [D, C_shard], scales by 8 and casts to
fp8e4; device computes 64*cos via fp8 DoubleRow matmuls and the
softmax denominator sum(exp(32cos-32)) per batch row. Fixed shift 32
replaces the row max (|logit|<=32), so no collective. Host does the
O(B) combine: margin-target correction, ln, weighted dot.

Device pipeline (v3): PSUM is the choke point (only ACT and DVE can
read it, ~1.1-1.15 ns/col each). PSUM is split into 4 independent
1024-col slots (2 banks each, bufs=1 tags), tiles strictly alternate
ACT/DVE so each engine owns a double-buffered pair of slots and the
PE's production (474ns/tile) hides under consumption (1.2-1.4us/tile):
 - ACT: exp activation + fused accum_out row-sum per tile.
 - DVE: Schraudolph fake-exp (affine to i16 bits = bf16(exp)) into a
   6-plane fi tile; Pool folds planes with two 2048-wide bf16 adds;
   DVE does one 2048-wide row-reduce per batch chunk.
All partial sums land in one [128, NBC, NSLOT] f32 tile, DMA'd out
once; the host sums slots and finishes the loss.
"""

import numpy as np
import ml_dtypes

import concourse.bass as bass
import concourse.tile as tile
from concourse import bacc, mybir
from concourse.bass_utils import run_bass_kernel_spmd

B = 512
D = 256
C = 100000
NCORES = 8
CSH = C // NCORES          # 12500 classes per core
CPAD = 12288               # 12*1024; tail classes handled on host
CTAIL = CSH - CPAD         # 212 tail classes per shard, on host

M0 = 0.5
M_MIN = 0.25
SCALE = 32.0
SHIFT = 32.0
FP8_PRESCALE = 8.0         # both operands scaled by 8 -> matmul gives 64*cos

LOG2E = 1.4426950408889634
FA = 64.0 * LOG2E
FB = 16256.0 - 4096.0 * LOG2E

f32 = mybir.dt.float32
bf16 = mybir.dt.bfloat16
i16 = mybir.dt.int16
fp8 = mybir.dt.float8e4

NBC = B // 128             # 4 batch chunks
NSLOT = 10                 # 7 ACT accum slots (6 wide + small) + 2 DVE

_cached_nc = None
_last_results = None


def _schraudolph_rho(fb):
    t = np.linspace(-60.0, -1.0, 200001)
    x = (t + 32.0) * 2.0
    y = np.float32(x) * np.float32(FA) + np.float32(fb)
    i = np.rint(y).astype(np.int16)
    v = i.view(ml_dtypes.bfloat16).astype(np.float64)
    return float(np.mean(v / np.exp(t)))


FB_EFF = FB - 128.0 * np.log2(_schraudolph_rho(FB))
FB_EFF = FB_EFF - 128.0 * np.log2(_schraudolph_rho(FB_EFF))


def _build():
    global _cached_nc
    if _cached_nc is not None:
        return _cached_nc

    nc = bacc.Bacc(
        "TRN2", target_bir_lowering=False, debug=False, num_devices=NCORES
    )

    wnT_d = nc.dram_tensor("wnT", [128, 2, CPAD], fp8, kind="ExternalInput")
    featnT_d = nc.dram_tensor("featnT", [128, 2, B], fp8, kind="ExternalInput")
    out_d = nc.dram_tensor("out", [128, NBC, NSLOT], f32, kind="ExternalOutput")

    with tile.TileContext(nc) as tc:
        with (
            tc.tile_pool(name="persist", bufs=1) as persist,
            tc.tile_pool(name="work", bufs=2) as work,
            tc.tile_pool(name="psum", bufs=1, space="PSUM") as psum,
        ):
            fsb = persist.tile([128, 2, B], fp8)
            nc.sync.dma_start(out=fsb[:], in_=featnT_d[:])

            wsb = persist.tile([128, 2, CPAD], fp8)
            # one 1024-col region per class-tile row; even rows on the
            # scalar queue, odd rows on sync, so consecutive rows arrive
            # in parallel and ahead of the row-major consumption order.
            # Row 0 is split in half so the very first matmul starts early.
            plan = [
                (nc.scalar, 0, 512),
                (nc.scalar, 512, 1024),
                (nc.scalar, 2048, 3072),
            ] + [
                (nc.sync, k * 1024, (k + 1) * 1024)
                for k in range(1, 12) if k != 2
            ]
            for eng, lo, hi in plan:
                eng.dma_start(out=wsb[:, :, lo:hi], in_=wnT_d[:, :, lo:hi])

            bias_s = persist.tile([128, 1], f32)
            nc.gpsimd.memset(bias_s[:], -SHIFT)
            tblw = persist.tile([128, 1], bf16)
            # dummy activation: hoists the Exp ACT_TABLE_LOAD into the DMA
            # window instead of serializing before the first real tile
            nc.scalar.activation(
                tblw[:], bias_s[:], mybir.ActivationFunctionType.Exp,
                bias=bias_s[:], scale=1.0,
            )

            S_out = persist.tile([128, NBC, NSLOT], f32)
            nc.gpsimd.memset(S_out[:], 0.0)

            esc = persist.tile([128, 1024], bf16)   # ACT dead-store target

            # Class-tile-major order: each weight region feeds all 4
            # batch chunks (4 LDW-rotated matmul groups), so the weight
            # DMA (~0.9 col/ns) stays ahead of consumption (~1.5 col/ns
            # across ACT+DVE). Position p in a row maps to PSUM slot p;
            # even positions are ACT tiles, odd are DVE, and the batch
            # chunks rotate so every chunk gets both engines. Row 11 is
            # all-ACT so the DVE fold chains drain under it.
            ROT_EVEN = [0, 1, 2, 3]
            ROT_ODD = [1, 0, 3, 2]

            fi6s = [
                work.tile([128, 6, 1024], i16, tag=f"fi6_{c}", bufs=1,
                          name=f"fi6_{c}")
                for c in range(NBC)
            ]
            taccs = [
                work.tile([128, 2, 1024], bf16, tag=f"tacc_{c}", bufs=1,
                          name=f"tacc_{c}")
                for c in range(NBC)
            ]
            a_slots = [0] * NBC
            d_cnts = [0] * NBC
            nd_of = [6, 6, 5, 6]
            # c0/c2 finish their D-planes by row 9: their folds all run on
            # Pool mid-kernel, DVE only does the final 512-fold + reduce.
            # c1/c3 finish at row 10: their last fold + merge run on DVE
            # immediately so the endgame is short.
            LATE = (1, 3)

            def emit_chain(c):
                tacc_c, fi6_c = taccs[c], fi6s[c]
                if c in LATE:
                    nc.vector.tensor_add(
                        tacc_c[:, 1, :], tacc_c[:, 1, :],
                        fi6_c[:, 5, :].bitcast(bf16),
                    )
                    nc.vector.tensor_add(
                        tacc_c[:, 0, :], tacc_c[:, 0, :], tacc_c[:, 1, :]
                    )
                else:
                    nc.gpsimd.tensor_add(
                        tacc_c[:, 0, :], tacc_c[:, 0, :], tacc_c[:, 1, :]
                    )
                nc.vector.tensor_add(
                    tacc_c[:, 0, 0:512], tacc_c[:, 0, 0:512],
                    tacc_c[:, 0, 512:1024],
                )
                nc.vector.tensor_reduce(
                    S_out[:, c, 8:9],
                    tacc_c[:, 0, 0:512],
                    axis=mybir.AxisListType.X,
                    op=mybir.AluOpType.add,
                )

            for ti in range(12):
                rot = ROT_EVEN if ti % 2 == 0 else ROT_ODD
                c0 = ti * 1024
                for p in range(4):
                    bc = rot[p]
                    is_act = (ti == 11) or (p % 2 == 0)
                    if ti == 0 and p == 0:
                        is_act = False   # extra D tile for c0 (balance)
                    lhs = fsb[:, :, bc * 128:(bc + 1) * 128]
                    ps = psum.tile([128, 1024], f32, tag=f"p{p}")
                    for j in (0, 512):
                        nc.tensor.matmul(
                            ps[:, j:j + 512],
                            lhs,
                            wsb[:, :, c0 + j:c0 + j + 512],
                            start=True, stop=True,
                            perf_mode=mybir.MatmulPerfMode.DoubleRow,
                        )
                    if is_act:
                        nc.scalar.activation(
                            esc[:], ps[:],
                            mybir.ActivationFunctionType.Exp,
                            bias=bias_s[:], scale=SCALE / (FP8_PRESCALE**2),
                            accum_out=S_out[:, bc, a_slots[bc]:a_slots[bc] + 1],
                        )
                        a_slots[bc] += 1
                    else:
                        k = d_cnts[bc]
                        fi6 = fi6s[bc]
                        tacc = taccs[bc]
                        nc.vector.tensor_scalar(
                            fi6[:, k, :], ps[:],
                            FA, FB_EFF,
                            mybir.AluOpType.mult, mybir.AluOpType.add,
                        )
                        d_cnts[bc] = k + 1
                        # incremental 1024-wide folds on Pool
                        if k == 1:
                            nc.gpsimd.tensor_add(
                                tacc[:, 0, :],
                                fi6[:, 0, :].bitcast(bf16),
                                fi6[:, 1, :].bitcast(bf16),
                            )
                        elif k == 3:
                            nc.gpsimd.tensor_add(
                                tacc[:, 1, :],
                                fi6[:, 2, :].bitcast(bf16),
                                fi6[:, 3, :].bitcast(bf16),
                            )
                        elif k == 4:
                            nc.gpsimd.tensor_add(
                                tacc[:, 0, :], tacc[:, 0, :],
                                fi6[:, 4, :].bitcast(bf16),
                            )
                        elif k == 5 and bc not in (1, 3):
                            nc.gpsimd.tensor_add(
                                tacc[:, 1, :], tacc[:, 1, :],
                                fi6[:, 5, :].bitcast(bf16),
                            )
                if ti == 10:
                    # c0 and c2 finished their D-planes by row 9
                    emit_chain(0)
                    emit_chain(2)
            emit_chain(1)
            emit_chain(3)

            nc.sync.dma_start(out=out_d[:], in_=S_out[:])

    nc.compile()
    _cached_nc = nc
    return nc


def _host_prep(features, weight, weights, labels):
    f = features.astype(np.float64)
    norms = np.sqrt((f * f).sum(axis=1))
    lo, hi = norms.min(), norms.max()
    denom = max(hi - lo, 1e-8)
    margins = np.clip(M_MIN + (M0 - M_MIN) * (norms - lo) / denom, M_MIN, M0)
    feat_n = f / np.maximum(norms, 1e-12)[:, None]

    wlab = weight[labels].astype(np.float64)
    wlab_n = wlab / np.maximum(
        np.sqrt((wlab * wlab).sum(axis=1)), 1e-12
    )[:, None]
    cos_t = np.clip((feat_n * wlab_n).sum(axis=1), -1.0 + 1e-7, 1.0 - 1e-7)
    cos_m = cos_t * np.cos(margins) - np.sqrt(1.0 - cos_t * cos_t) * np.sin(
        margins
    )
    t_logit = SCALE * cos_m
    corr = np.exp(SCALE * cos_m - SHIFT) - np.exp(SCALE * cos_t - SHIFT)
    coef = weights.astype(np.float64) / B
    return feat_n, corr, coef, t_logit


def _to_dr_layout(mat_t, width):
    """[D, X] f32 -> [128, 2, X] fp8 with k = j*128 + p."""
    a = mat_t.reshape(2, 128, width)          # [j, p, X]
    a = np.ascontiguousarray(a.transpose(1, 0, 2))  # [p, j, X]
    return a.astype(ml_dtypes.float8_e4m3)


def kernel(features, weight, weights, labels):
    global _last_results
    features = np.asarray(features, dtype=np.float32)
    weight = np.asarray(weight, dtype=np.float32)
    weights = np.asarray(weights, dtype=np.float32)
    labels = np.asarray(labels).astype(np.int64)

    feat_n, corr, coef, t_logit = _host_prep(features, weight, weights, labels)

    wn = weight / np.maximum(
        np.linalg.norm(weight, axis=1, keepdims=True), 1e-12
    )
    featnT = np.ascontiguousarray(feat_n.T.astype(np.float32)) * FP8_PRESCALE
    featnT8 = _to_dr_layout(featnT, B)

    in_maps = []
    tail_rows = []
    for i in range(NCORES):
        sh = wn[i * CSH:(i + 1) * CSH]  # [CSH, D]
        wt = np.ascontiguousarray(sh[:CPAD].T.astype(np.float32)) * FP8_PRESCALE
        in_maps.append(
            {"wnT": _to_dr_layout(wt, CPAD), "featnT": featnT8}
        )
        tail_rows.append(sh[CPAD:])
    # exact host contribution of the 212 tail classes per shard
    wtail = np.concatenate(tail_rows, axis=0)          # [8*212, D]
    cos_tail = feat_n @ wtail.T                        # [B, 1696] f64
    S_tail = np.exp(SCALE * cos_tail - SHIFT).sum(axis=1)

    nc = _build()
    res = run_bass_kernel_spmd(nc, in_maps, list(range(NCORES)))
    _last_results = res

    S = np.zeros(B, dtype=np.float64)
    for i in range(NCORES):
        sc = np.asarray(res.results[i]["out"], dtype=np.float64)
        for bc in range(NBC):
            S[bc * 128:(bc + 1) * 128] += sc[:, bc, :].sum(axis=1)

    Z = S + S_tail + corr
    per = SHIFT + np.log(Z) - t_logit
    loss = float((coef * per).sum())
    return np.array(loss, dtype=np.float32)


# revision 14
# speedup vs baseline: 1.0607x; 1.0607x over previous
"""AdaFace loss on 8 TRN2 NeuronCores, class-parallel.

Strategy: shard the 100k weight rows (classes) across 8 cores. Host
pre-normalizes rows, transposes to [D, C_shard], scales by 8 and casts to
fp8e4; device computes 64*cos via fp8 DoubleRow matmuls and the
softmax denominator sum(exp(32cos-32)) per batch row. Fixed shift 32
replaces the row max (|logit|<=32), so no collective. Host does the
O(B) combine: margin-target correction, ln, weighted dot.

Device pipeline (v3): PSUM is the choke point (only ACT and DVE can
read it, ~1.1-1.15 ns/col each). PSUM is split into 4 independent
1024-col slots (2 banks each, bufs=1 tags), tiles strictly alternate
ACT/DVE so each engine owns a double-buffered pair of slots and the
PE's production (474ns/tile) hides under consumption (1.2-1.4us/tile):
 - ACT: exp activation + fused accum_out row-sum per tile.
 - DVE: Schraudolph fake-exp (affine to i16 bits = bf16(exp)) into a
   6-plane fi tile; Pool folds planes with two 2048-wide bf16 adds;
   DVE does one 2048-wide row-reduce per batch chunk.
All partial sums land in one [128, NBC, NSLOT] f32 tile, DMA'd out
once; the host sums slots and finishes the loss.
"""

import numpy as np
import ml_dtypes

import concourse.bass as bass
import concourse.tile as tile
from concourse import bacc, mybir
from concourse.bass_utils import run_bass_kernel_spmd

B = 512
D = 256
C = 100000
NCORES = 8
CSH = C // NCORES          # 12500 classes per core
CPAD = 12288               # 12*1024; tail classes handled on host
CTAIL = CSH - CPAD         # 212 tail classes per shard, on host

M0 = 0.5
M_MIN = 0.25
SCALE = 32.0
SHIFT = 32.0
FP8_PRESCALE = 8.0         # both operands scaled by 8 -> matmul gives 64*cos

LOG2E = 1.4426950408889634
FA = 64.0 * LOG2E
FB = 16256.0 - 4096.0 * LOG2E

f32 = mybir.dt.float32
bf16 = mybir.dt.bfloat16
i16 = mybir.dt.int16
fp8 = mybir.dt.float8e4

NBC = B // 128             # 4 batch chunks
NSLOT = 10                 # 7 ACT accum slots (6 wide + small) + 2 DVE

_cached_nc = None
_last_results = None


def _schraudolph_rho(fb):
    t = np.linspace(-60.0, -1.0, 200001)
    x = (t + 32.0) * 2.0
    y = np.float32(x) * np.float32(FA) + np.float32(fb)
    i = np.rint(y).astype(np.int16)
    v = i.view(ml_dtypes.bfloat16).astype(np.float64)
    return float(np.mean(v / np.exp(t)))


FB_EFF = FB - 128.0 * np.log2(_schraudolph_rho(FB))
FB_EFF = FB_EFF - 128.0 * np.log2(_schraudolph_rho(FB_EFF))


def _build():
    global _cached_nc
    if _cached_nc is not None:
        return _cached_nc

    nc = bacc.Bacc(
        "TRN2", target_bir_lowering=False, debug=False, num_devices=NCORES
    )

    wnT_d = nc.dram_tensor("wnT", [128, 2, CPAD], fp8, kind="ExternalInput")
    featnT_d = nc.dram_tensor("featnT", [128, 2, B], fp8, kind="ExternalInput")
    out_d = nc.dram_tensor("out", [128, NBC, NSLOT], f32, kind="ExternalOutput")

    with tile.TileContext(nc) as tc:
        with (
            tc.tile_pool(name="persist", bufs=1) as persist,
            tc.tile_pool(name="work", bufs=2) as work,
            tc.tile_pool(name="psum", bufs=1, space="PSUM") as psum,
        ):
            fsb = persist.tile([128, 2, B], fp8)
            nc.sync.dma_start(out=fsb[:], in_=featnT_d[:])

            bias_s = persist.tile([128, 1], f32)
            nc.gpsimd.memset(bias_s[:], -SHIFT)
            tblw = persist.tile([128, 1], bf16)
            # dummy activation: hoists the Exp ACT_TABLE_LOAD before the
            # DMA ring instructions so it overlaps the weight transfers
            nc.scalar.activation(
                tblw[:], bias_s[:], mybir.ActivationFunctionType.Exp,
                bias=bias_s[:], scale=1.0,
            )

            wsb = persist.tile([128, 2, CPAD], fp8)
            # one 1024-col region per class-tile row; even rows on the
            # scalar queue, odd rows on sync, so consecutive rows arrive
            # in parallel and ahead of the row-major consumption order.
            # Row 0 is split in half so the very first matmul starts early.
            plan = [
                (nc.scalar, 0, 512),
                (nc.scalar, 512, 1024),
                (nc.scalar, 2048, 3072),
            ] + [
                (nc.sync, k * 1024, (k + 1) * 1024)
                for k in range(1, 12) if k != 2
            ]
            for eng, lo, hi in plan:
                eng.dma_start(out=wsb[:, :, lo:hi], in_=wnT_d[:, :, lo:hi])

            S_out = persist.tile([128, NBC, NSLOT], f32)
            nc.gpsimd.memset(S_out[:], 0.0)

            esc = persist.tile([128, 1024], bf16)   # ACT dead-store target

            # Class-tile-major order: each weight region feeds all 4
            # batch chunks (4 LDW-rotated matmul groups), so the weight
            # DMA (~0.9 col/ns) stays ahead of consumption (~1.5 col/ns
            # across ACT+DVE). Position p in a row maps to PSUM slot p;
            # even positions are ACT tiles, odd are DVE, and the batch
            # chunks rotate so every chunk gets both engines. Row 11 is
            # all-ACT so the DVE fold chains drain under it.
            ROT_EVEN = [0, 1, 2, 3]
            ROT_ODD = [1, 0, 3, 2]

            fi6s = [
                work.tile([128, 6, 1024], i16, tag=f"fi6_{c}", bufs=1,
                          name=f"fi6_{c}")
                for c in range(NBC)
            ]
            taccs = [
                work.tile([128, 2, 1024], bf16, tag=f"tacc_{c}", bufs=1,
                          name=f"tacc_{c}")
                for c in range(NBC)
            ]
            a_slots = [0] * NBC
            d_cnts = [0] * NBC
            nd_of = [6, 6, 5, 6]
            # c0/c2 finish their D-planes by row 9: their folds all run on
            # Pool mid-kernel, DVE only does the final 512-fold + reduce.
            # c1/c3 finish at row 10: their last fold + merge run on DVE
            # immediately so the endgame is short.
            LATE = (1, 3)

            def emit_chain(c):
                tacc_c, fi6_c = taccs[c], fi6s[c]
                if c in LATE:
                    nc.vector.tensor_add(
                        tacc_c[:, 1, :], tacc_c[:, 1, :],
                        fi6_c[:, 5, :].bitcast(bf16),
                    )
                elif nd_of[c] == 5:
                    nc.vector.tensor_add(
                        tacc_c[:, 0, :], tacc_c[:, 0, :],
                        fi6_c[:, 4, :].bitcast(bf16),
                    )
                nc.vector.tensor_add(
                    tacc_c[:, 0, :], tacc_c[:, 0, :], tacc_c[:, 1, :]
                )
                nc.vector.tensor_add(
                    tacc_c[:, 0, 0:512], tacc_c[:, 0, 0:512],
                    tacc_c[:, 0, 512:1024],
                )
                nc.vector.tensor_reduce(
                    S_out[:, c, 8:9],
                    tacc_c[:, 0, 0:512],
                    axis=mybir.AxisListType.X,
                    op=mybir.AluOpType.add,
                )

            for ti in range(12):
                rot = ROT_EVEN if ti % 2 == 0 else ROT_ODD
                c0 = ti * 1024
                for p in range(4):
                    bc = rot[p]
                    is_act = (ti == 11) or (p % 2 == 0)
                    if ti == 0 and p == 0:
                        is_act = False   # extra D tile for c0 (balance)
                    lhs = fsb[:, :, bc * 128:(bc + 1) * 128]
                    ps = psum.tile([128, 1024], f32, tag=f"p{p}")
                    for j in (0, 512):
                        nc.tensor.matmul(
                            ps[:, j:j + 512],
                            lhs,
                            wsb[:, :, c0 + j:c0 + j + 512],
                            start=True, stop=True,
                            perf_mode=mybir.MatmulPerfMode.DoubleRow,
                        )
                    if is_act:
                        nc.scalar.activation(
                            esc[:], ps[:],
                            mybir.ActivationFunctionType.Exp,
                            bias=bias_s[:], scale=SCALE / (FP8_PRESCALE**2),
                            accum_out=S_out[:, bc, a_slots[bc]:a_slots[bc] + 1],
                        )
                        a_slots[bc] += 1
                    else:
                        k = d_cnts[bc]
                        fi6 = fi6s[bc]
                        tacc = taccs[bc]
                        nc.vector.tensor_scalar(
                            fi6[:, k, :], ps[:],
                            FA, FB_EFF,
                            mybir.AluOpType.mult, mybir.AluOpType.add,
                        )
                        d_cnts[bc] = k + 1
                        # incremental 1024-wide folds on Pool
                        if k == 1:
                            nc.gpsimd.tensor_add(
                                tacc[:, 0, :],
                                fi6[:, 0, :].bitcast(bf16),
                                fi6[:, 1, :].bitcast(bf16),
                            )
                        elif k == 3:
                            nc.gpsimd.tensor_add(
                                tacc[:, 1, :],
                                fi6[:, 2, :].bitcast(bf16),
                                fi6[:, 3, :].bitcast(bf16),
                            )
                        elif k == 4 and nd_of[bc] == 6:
                            nc.gpsimd.tensor_add(
                                tacc[:, 0, :], tacc[:, 0, :],
                                fi6[:, 4, :].bitcast(bf16),
                            )
                        elif k == 5 and bc not in (1, 3):
                            nc.gpsimd.tensor_add(
                                tacc[:, 1, :], tacc[:, 1, :],
                                fi6[:, 5, :].bitcast(bf16),
                            )
                if ti == 10:
                    # c0 and c2 finished their D-planes by row 9
                    emit_chain(0)
                    emit_chain(2)
            emit_chain(1)
            emit_chain(3)

            nc.sync.dma_start(out=out_d[:], in_=S_out[:])

    nc.compile()
    _cached_nc = nc
    return nc


def _host_prep(features, weight, weights, labels):
    f = features.astype(np.float64)
    norms = np.sqrt((f * f).sum(axis=1))
    lo, hi = norms.min(), norms.max()
    denom = max(hi - lo, 1e-8)
    margins = np.clip(M_MIN + (M0 - M_MIN) * (norms - lo) / denom, M_MIN, M0)
    feat_n = f / np.maximum(norms, 1e-12)[:, None]

    wlab = weight[labels].astype(np.float64)
    wlab_n = wlab / np.maximum(
        np.sqrt((wlab * wlab).sum(axis=1)), 1e-12
    )[:, None]
    cos_t = np.clip((feat_n * wlab_n).sum(axis=1), -1.0 + 1e-7, 1.0 - 1e-7)
    cos_m = cos_t * np.cos(margins) - np.sqrt(1.0 - cos_t * cos_t) * np.sin(
        margins
    )
    t_logit = SCALE * cos_m
    corr = np.exp(SCALE * cos_m - SHIFT) - np.exp(SCALE * cos_t - SHIFT)
    coef = weights.astype(np.float64) / B
    return feat_n, corr, coef, t_logit


def _to_dr_layout(mat_t, width):
    """[D, X] f32 -> [128, 2, X] fp8 with k = j*128 + p."""
    a = mat_t.reshape(2, 128, width)          # [j, p, X]
    a = np.ascontiguousarray(a.transpose(1, 0, 2))  # [p, j, X]
    return a.astype(ml_dtypes.float8_e4m3)


def kernel(features, weight, weights, labels):
    global _last_results
    features = np.asarray(features, dtype=np.float32)
    weight = np.asarray(weight, dtype=np.float32)
    weights = np.asarray(weights, dtype=np.float32)
    labels = np.asarray(labels).astype(np.int64)

    feat_n, corr, coef, t_logit = _host_prep(features, weight, weights, labels)

    wn = weight / np.maximum(
        np.linalg.norm(weight, axis=1, keepdims=True), 1e-12
    )
    featnT = np.ascontiguousarray(feat_n.T.astype(np.float32)) * FP8_PRESCALE
    featnT8 = _to_dr_layout(featnT, B)

    in_maps = []
    tail_rows = []
    for i in range(NCORES):
        sh = wn[i * CSH:(i + 1) * CSH]  # [CSH, D]
        wt = np.ascontiguousarray(sh[:CPAD].T.astype(np.float32)) * FP8_PRESCALE
        in_maps.append(
            {"wnT": _to_dr_layout(wt, CPAD), "featnT": featnT8}
        )
        tail_rows.append(sh[CPAD:])
    # exact host contribution of the 212 tail classes per shard
    wtail = np.concatenate(tail_rows, axis=0)          # [8*212, D]
    cos_tail = feat_n @ wtail.T                        # [B, 1696] f64
    S_tail = np.exp(SCALE * cos_tail - SHIFT).sum(axis=1)

    nc = _build()
    res = run_bass_kernel_spmd(nc, in_maps, list(range(NCORES)))
    _last_results = res

    S = np.zeros(B, dtype=np.float64)
    for i in range(NCORES):
        sc = np.asarray(res.results[i]["out"], dtype=np.float64)
        for bc in range(NBC):
            S[bc * 128:(bc + 1) * 128] += sc[:, bc, :].sum(axis=1)

    Z = S + S_tail + corr
    per = SHIFT + np.log(Z) - t_logit
    loss = float((coef * per).sum())
    return np.array(loss, dtype=np.float32)


# revision 15
# speedup vs baseline: 1.1100x; 1.0465x over previous
"""AdaFace loss on 8 TRN2 NeuronCores, class-parallel.

Strategy: shard the 100k weight rows (classes) across 8 cores. Host
pre-normalizes rows, transposes to [D, C_shard], scales by 8 and casts to
fp8e4; device computes 64*cos via fp8 DoubleRow matmuls and the
softmax denominator sum(exp(32cos-32)) per batch row. Fixed shift 32
replaces the row max (|logit|<=32), so no collective. Host does the
O(B) combine: margin-target correction, ln, weighted dot.

Device pipeline (v3): PSUM is the choke point (only ACT and DVE can
read it, ~1.1-1.15 ns/col each). PSUM is split into 4 independent
1024-col slots (2 banks each, bufs=1 tags), tiles strictly alternate
ACT/DVE so each engine owns a double-buffered pair of slots and the
PE's production (474ns/tile) hides under consumption (1.2-1.4us/tile):
 - ACT: exp activation + fused accum_out row-sum per tile.
 - DVE: Schraudolph fake-exp (affine to i16 bits = bf16(exp)) into a
   6-plane fi tile; Pool folds planes with two 2048-wide bf16 adds;
   DVE does one 2048-wide row-reduce per batch chunk.
All partial sums land in one [128, NBC, NSLOT] f32 tile, DMA'd out
once; the host sums slots and finishes the loss.
"""

import numpy as np
import ml_dtypes

import concourse.bass as bass
import concourse.tile as tile
from concourse import bacc, mybir
from concourse.bass_utils import run_bass_kernel_spmd

B = 512
D = 256
C = 100000
NCORES = 8
CSH = C // NCORES          # 12500 classes per core
CPAD = 12288               # 12*1024; tail classes handled on host
CTAIL = CSH - CPAD         # 212 tail classes per shard, on host

M0 = 0.5
M_MIN = 0.25
SCALE = 32.0
SHIFT = 32.0
FP8_PRESCALE = 8.0         # both operands scaled by 8 -> matmul gives 64*cos

LOG2E = 1.4426950408889634
FA = 64.0 * LOG2E
FB = 16256.0 - 4096.0 * LOG2E

f32 = mybir.dt.float32
bf16 = mybir.dt.bfloat16
i16 = mybir.dt.int16
fp8 = mybir.dt.float8e4

NBC = B // 128             # 4 batch chunks
NSLOT = 10                 # 7 ACT accum slots (6 wide + small) + 2 DVE

_cached_nc = None
_last_results = None


def _schraudolph_rho(fb):
    t = np.linspace(-60.0, -1.0, 200001)
    x = (t + 32.0) * 2.0
    y = np.float32(x) * np.float32(FA) + np.float32(fb)
    i = np.rint(y).astype(np.int16)
    v = i.view(ml_dtypes.bfloat16).astype(np.float64)
    return float(np.mean(v / np.exp(t)))


FB_EFF = FB - 128.0 * np.log2(_schraudolph_rho(FB))
FB_EFF = FB_EFF - 128.0 * np.log2(_schraudolph_rho(FB_EFF))


def _build():
    global _cached_nc
    if _cached_nc is not None:
        return _cached_nc

    nc = bacc.Bacc(
        "TRN2", target_bir_lowering=False, debug=False, num_devices=NCORES
    )

    wnT_d = nc.dram_tensor("wnT", [128, 2, CPAD], fp8, kind="ExternalInput")
    featnT_d = nc.dram_tensor("featnT", [128, 2, B], fp8, kind="ExternalInput")
    out_d = nc.dram_tensor("out", [128, NBC, NSLOT], f32, kind="ExternalOutput")

    with tile.TileContext(nc) as tc:
        with (
            tc.tile_pool(name="persist", bufs=1) as persist,
            tc.tile_pool(name="work", bufs=2) as work,
            tc.tile_pool(name="psum", bufs=1, space="PSUM") as psum,
        ):
            fsb = persist.tile([128, 2, B], fp8)
            nc.sync.dma_start(out=fsb[:], in_=featnT_d[:])

            bias_s = persist.tile([128, 1], f32)
            nc.gpsimd.memset(bias_s[:], -SHIFT)
            tblw = persist.tile([128, 1], bf16)
            # dummy activation: hoists the Exp ACT_TABLE_LOAD before the
            # DMA ring instructions so it overlaps the weight transfers
            nc.scalar.activation(
                tblw[:], bias_s[:], mybir.ActivationFunctionType.Exp,
                bias=bias_s[:], scale=1.0,
            )

            wsb = persist.tile([128, 2, CPAD], fp8)
            # one 1024-col region per class-tile row; even rows on the
            # scalar queue, odd rows on sync, so consecutive rows arrive
            # in parallel and ahead of the row-major consumption order.
            # Row 0 is split in half so the very first matmul starts early.
            plan = [
                (nc.scalar, 0, 512),
                (nc.scalar, 512, 1024),
                (nc.scalar, 2048, 3072),
            ] + [
                (nc.sync, k * 1024, (k + 1) * 1024)
                for k in range(1, 12) if k != 2
            ]
            for eng, lo, hi in plan:
                eng.dma_start(out=wsb[:, :, lo:hi], in_=wnT_d[:, :, lo:hi])

            S_out = persist.tile([128, NBC, NSLOT], f32)
            nc.gpsimd.memset(S_out[:], 0.0)

            esc = persist.tile([128, 1024], bf16)   # ACT dead-store target

            # Class-tile-major order: each weight region feeds all 4
            # batch chunks (4 LDW-rotated matmul groups), so the weight
            # DMA (~0.9 col/ns) stays ahead of consumption (~1.5 col/ns
            # across ACT+DVE). Position p in a row maps to PSUM slot p;
            # even positions are ACT tiles, odd are DVE, and the batch
            # chunks rotate so every chunk gets both engines. Row 11 is
            # all-ACT so the DVE fold chains drain under it.
            ROT_EVEN = [0, 1, 2, 3]
            ROT_ODD = [1, 0, 3, 2]

            fi6s = [
                work.tile([128, 6, 1024], i16, tag=f"fi6_{c}", bufs=1,
                          name=f"fi6_{c}")
                for c in range(NBC)
            ]
            taccs = [
                work.tile([128, 2, 1024], bf16, tag=f"tacc_{c}", bufs=1,
                          name=f"tacc_{c}")
                for c in range(NBC)
            ]
            a_slots = [0] * NBC
            d_cnts = [0] * NBC
            nd_of = [5, 6, 5, 6]
            # c0/c2 finish their D-planes by row 9: their folds all run on
            # Pool mid-kernel, DVE only does the final 512-fold + reduce.
            # c1/c3 finish at row 10: their last fold + merge run on DVE
            # immediately so the endgame is short.
            LATE = (1, 3)

            def emit_chain(c):
                tacc_c, fi6_c = taccs[c], fi6s[c]
                if c in LATE:
                    nc.vector.tensor_add(
                        tacc_c[:, 1, :], tacc_c[:, 1, :],
                        fi6_c[:, 5, :].bitcast(bf16),
                    )
                elif nd_of[c] == 5:
                    nc.vector.tensor_add(
                        tacc_c[:, 0, :], tacc_c[:, 0, :],
                        fi6_c[:, 4, :].bitcast(bf16),
                    )
                nc.vector.tensor_add(
                    tacc_c[:, 0, :], tacc_c[:, 0, :], tacc_c[:, 1, :]
                )
                nc.vector.tensor_add(
                    tacc_c[:, 0, 0:512], tacc_c[:, 0, 0:512],
                    tacc_c[:, 0, 512:1024],
                )
                nc.vector.tensor_reduce(
                    S_out[:, c, 8:9],
                    tacc_c[:, 0, 0:512],
                    axis=mybir.AxisListType.X,
                    op=mybir.AluOpType.add,
                )

            for ti in range(12):
                rot = ROT_EVEN if ti % 2 == 0 else ROT_ODD
                c0 = ti * 1024
                for p in range(4):
                    bc = rot[p]
                    # row 0 is all-ACT (only needs the first DMA chunks),
                    # row 1 all-DVE, row 11 all-ACT (DVE chains drain under
                    # it); middle rows alternate by position
                    if ti == 0 or ti == 11:
                        is_act = True
                    elif ti == 1:
                        is_act = False
                    else:
                        is_act = (p % 2 == 0)
                    lhs = fsb[:, :, bc * 128:(bc + 1) * 128]
                    ps = psum.tile([128, 1024], f32, tag=f"p{p}")
                    for j in (0, 512):
                        nc.tensor.matmul(
                            ps[:, j:j + 512],
                            lhs,
                            wsb[:, :, c0 + j:c0 + j + 512],
                            start=True, stop=True,
                            perf_mode=mybir.MatmulPerfMode.DoubleRow,
                        )
                    if is_act:
                        nc.scalar.activation(
                            esc[:], ps[:],
                            mybir.ActivationFunctionType.Exp,
                            bias=bias_s[:], scale=SCALE / (FP8_PRESCALE**2),
                            accum_out=S_out[:, bc, a_slots[bc]:a_slots[bc] + 1],
                        )
                        a_slots[bc] += 1
                    else:
                        k = d_cnts[bc]
                        fi6 = fi6s[bc]
                        tacc = taccs[bc]
                        nc.vector.tensor_scalar(
                            fi6[:, k, :], ps[:],
                            FA, FB_EFF,
                            mybir.AluOpType.mult, mybir.AluOpType.add,
                        )
                        d_cnts[bc] = k + 1
                        # incremental 1024-wide folds on Pool
                        if k == 1:
                            nc.gpsimd.tensor_add(
                                tacc[:, 0, :],
                                fi6[:, 0, :].bitcast(bf16),
                                fi6[:, 1, :].bitcast(bf16),
                            )
                        elif k == 3:
                            nc.gpsimd.tensor_add(
                                tacc[:, 1, :],
                                fi6[:, 2, :].bitcast(bf16),
                                fi6[:, 3, :].bitcast(bf16),
                            )
                        elif k == 4 and nd_of[bc] == 6:
                            nc.gpsimd.tensor_add(
                                tacc[:, 0, :], tacc[:, 0, :],
                                fi6[:, 4, :].bitcast(bf16),
                            )
                        elif k == 5 and bc not in (1, 3):
                            nc.gpsimd.tensor_add(
                                tacc[:, 1, :], tacc[:, 1, :],
                                fi6[:, 5, :].bitcast(bf16),
                            )
                if ti == 10:
                    # c0 and c2 finished their D-planes by row 9
                    emit_chain(0)
                    emit_chain(2)
            emit_chain(1)
            emit_chain(3)

            nc.sync.dma_start(out=out_d[:], in_=S_out[:])

    nc.compile()
    _cached_nc = nc
    return nc


def _host_prep(features, weight, weights, labels):
    f = features.astype(np.float64)
    norms = np.sqrt((f * f).sum(axis=1))
    lo, hi = norms.min(), norms.max()
    denom = max(hi - lo, 1e-8)
    margins = np.clip(M_MIN + (M0 - M_MIN) * (norms - lo) / denom, M_MIN, M0)
    feat_n = f / np.maximum(norms, 1e-12)[:, None]

    wlab = weight[labels].astype(np.float64)
    wlab_n = wlab / np.maximum(
        np.sqrt((wlab * wlab).sum(axis=1)), 1e-12
    )[:, None]
    cos_t = np.clip((feat_n * wlab_n).sum(axis=1), -1.0 + 1e-7, 1.0 - 1e-7)
    cos_m = cos_t * np.cos(margins) - np.sqrt(1.0 - cos_t * cos_t) * np.sin(
        margins
    )
    t_logit = SCALE * cos_m
    corr = np.exp(SCALE * cos_m - SHIFT) - np.exp(SCALE * cos_t - SHIFT)
    coef = weights.astype(np.float64) / B
    return feat_n, corr, coef, t_logit


def _to_dr_layout(mat_t, width):
    """[D, X] f32 -> [128, 2, X] fp8 with k = j*128 + p."""
    a = mat_t.reshape(2, 128, width)          # [j, p, X]
    a = np.ascontiguousarray(a.transpose(1, 0, 2))  # [p, j, X]
    return a.astype(ml_dtypes.float8_e4m3)


def kernel(features, weight, weights, labels):
    global _last_results
    features = np.asarray(features, dtype=np.float32)
    weight = np.asarray(weight, dtype=np.float32)
    weights = np.asarray(weights, dtype=np.float32)
    labels = np.asarray(labels).astype(np.int64)

    feat_n, corr, coef, t_logit = _host_prep(features, weight, weights, labels)

    wn = weight / np.maximum(
        np.linalg.norm(weight, axis=1, keepdims=True), 1e-12
    )
    featnT = np.ascontiguousarray(feat_n.T.astype(np.float32)) * FP8_PRESCALE
    featnT8 = _to_dr_layout(featnT, B)

    in_maps = []
    tail_rows = []
    for i in range(NCORES):
        sh = wn[i * CSH:(i + 1) * CSH]  # [CSH, D]
        wt = np.ascontiguousarray(sh[:CPAD].T.astype(np.float32)) * FP8_PRESCALE
        in_maps.append(
            {"wnT": _to_dr_layout(wt, CPAD), "featnT": featnT8}
        )
        tail_rows.append(sh[CPAD:])
    # exact host contribution of the 212 tail classes per shard
    wtail = np.concatenate(tail_rows, axis=0)          # [8*212, D]
    cos_tail = feat_n @ wtail.T                        # [B, 1696] f64
    S_tail = np.exp(SCALE * cos_tail - SHIFT).sum(axis=1)

    nc = _build()
    res = run_bass_kernel_spmd(nc, in_maps, list(range(NCORES)))
    _last_results = res

    S = np.zeros(B, dtype=np.float64)
    for i in range(NCORES):
        sc = np.asarray(res.results[i]["out"], dtype=np.float64)
        for bc in range(NBC):
            S[bc * 128:(bc + 1) * 128] += sc[:, bc, :].sum(axis=1)

    Z = S + S_tail + corr
    per = SHIFT + np.log(Z) - t_logit
    loss = float((coef * per).sum())
    return np.array(loss, dtype=np.float32)
